# revision 9
# baseline (speedup 1.0000x reference)
"""GNN message-passing kernel for Trainium2 (8 NeuronCores).

Reference computation:
    out[b,i,f] = X[b,0,i,i,f] + sum_{k=1..3} sum_j A[b,i,j] * X[b,k,i,j,f]

Sharding: 8 cores = (batch b in 0..3) x (i-half h in 0..1); each core owns
a (b, 128-row i-slab) of the output. Hop 0 only contributes its diagonal,
so only X[b,1:4] (3/4 of X) plus the hop-0 diagonal rows are ever sent to
the device. X is converted to bf16 on the host (rel tol is 2e-2; bf16
round-to-nearest gives ~5e-3), halving DMA traffic to ~12.6 MB per core,
and re-laid-out chunk-major so each j-chunk is one fully contiguous DMA.

Per-core device kernel (chunks of CJ j-columns, small first/last):
  - One DMA per chunk: [128 part x (3 hops * CJ * F)] contiguous.
  - Hop sum on the TensorEngine: identity-stationary bf16 matmuls (single
    pass, 1024-col moving) accumulate x1+x2+x3 into PSUM fp32, after a
    HAM warm-up burst sized to bridge into chunk 0 (no >3.4us PE gap).
  - DVE: broadcast-AP multiply by A[i,j] (PSUM fp32 src, 1x mode) writing
    bf16; log-tree of bf16 tensor_adds (2x mode) for the j-reduction;
    per-chunk partial into a running [128, 4*F] accumulator so the final
    tail is short.
"""

import sys

if "/opt/trn_rl_repo" not in sys.path:
    sys.path.insert(0, "/opt/trn_rl_repo")

import ml_dtypes
import numpy as np

import concourse.bacc as bacc
import concourse.bass as bass
import concourse.mybir as mybir
from concourse.bass_utils import run_bass_kernel_spmd
from concourse.tile import TileContext

BATCH, KP1, N, F = 4, 4, 256, 64
NH = N // 2          # 128 rows of output per core (partition dim)
CJS = [16, 16, 32, 32, 32, 32, 32, 32, 16, 16]   # sum = 256
assert sum(CJS) == N
MMCOL = 512          # moving columns per matmul (ISA max per s3d3 check)
FP32 = mybir.dt.float32
BF16 = mybir.dt.bfloat16
BF16_NP = ml_dtypes.bfloat16

_CACHE = {}


def _build_nc():
    if "nc" in _CACHE:
        return _CACHE["nc"]
    nc = bacc.Bacc("TRN2", target_bir_lowering=False, debug=False, num_devices=8)
    # chunk-major: all of chunk c (3 hops x 128 i x CJ j x F) contiguous
    xk = nc.dram_tensor("xk", [NH * 3 * N * F], BF16, kind="ExternalInput").ap()
    a = nc.dram_tensor("a", [NH, N], FP32, kind="ExternalInput").ap()
    d = nc.dram_tensor("d", [NH, F], FP32, kind="ExternalInput").ap()
    eye = nc.dram_tensor("eye", [128, 128], BF16, kind="ExternalInput").ap()
    out = nc.dram_tensor("out", [NH, F], FP32, kind="ExternalOutput").ap()

    with TileContext(nc) as tc:
        with (
            tc.tile_pool(name="const", bufs=1) as cpool,
            tc.tile_pool(name="xs", bufs=4) as xpool,
            tc.tile_pool(name="pr", bufs=2) as prpool,
            tc.tile_pool(name="ac", bufs=1) as acpool,
            tc.tile_pool(name="ps", bufs=2, space="PSUM") as pspool,
        ):
            eye_sb = cpool.tile([128, 128], BF16)
            nc.sync.dma_start(out=eye_sb[:, :], in_=eye[:, :])
            a_sb = cpool.tile([128, N], FP32)
            d_sb = cpool.tile([128, F], FP32)

            acc = acpool.tile([128, 4 * F], BF16)   # running 4-j-group sums
            accf = acpool.tile([128, F], FP32)

            # PE warm-up: bridge from eye-DMA arrival to chunk-0 compute
            # with cold 128-col matmuls so HAM is warming before the real
            # stream. The first matmul self-loads the identity stationary;
            # every later matmul sets ldweights=False (bf16 path supports
            # non-self-loading MMs) so the PE never reloads it -- this
            # removes a serialized ~110ns LDWEIGHTS per matmul.
            warm = pspool.tile([128, max(CJS) * F], FP32, name="ps", tag="ps")
            for w in range(24):
                mm = nc.tensor.matmul(
                    warm[:, 0:128],
                    eye_sb[:, :],
                    eye_sb[:, :],
                    start=True,
                    stop=True,
                )
                if w > 0:
                    mm.ldweights = False

            first = True
            xoff = 0
            for c, CJ in enumerate(CJS):
                CF = CJ * F
                xt = xpool.tile([128, 3 * CF], BF16, name="xt", tag="xt")
                src = bass.AP(xk.tensor, xoff, [[3 * CF, 128], [1, 3 * CF]])
                nc.sync.dma_start(out=xt[:, :], in_=src)
                xoff += 128 * 3 * CF
                if c == 0:
                    # a/d triggers queue behind chunk 0's (needed later)
                    nc.sync.dma_start(out=a_sb[:, :], in_=a[:, :])
                    nc.sync.dma_start(out=d_sb[:, :], in_=d[:, :])

                # hop sum on TensorEngine: bf16 identity matmuls, PSUM fp32
                ps = pspool.tile([128, CF], FP32, name="ps", tag="ps")
                for s in range(CF // MMCOL):
                    sl = slice(s * MMCOL, (s + 1) * MMCOL)
                    for k in range(3):
                        mm = nc.tensor.matmul(
                            ps[:, sl],
                            eye_sb[:, :],
                            xt[:, k * CF + s * MMCOL : k * CF + (s + 1) * MMCOL],
                            start=(k == 0),
                            stop=(k == 2),
                        )
                        mm.ldweights = False

                # prod[i, j*F+f] = ps[i, j*F+f] * a_sb[i, j0+j]  (bf16 out)
                j0 = sum(CJS[:c])
                prod = prpool.tile([128, CF], BF16, name="prod", tag="prod")
                ps_step = ps.ap[0][0]
                pr_step = prod.ap[0][0]
                a_step = a_sb.ap[0][0]
                in0 = bass.AP(ps.tensor, 0, [[ps_step, 128], [F, CJ], [1, F]])
                in1 = bass.AP(a_sb.tensor, j0, [[a_step, 128], [1, CJ], [0, F]])
                po = bass.AP(prod.tensor, 0, [[pr_step, 128], [F, CJ], [1, F]])
                nc.vector.tensor_mul(po, in0, in1)

                # j-reduction tree (bf16 2x mode) down to 4 j-groups, then
                # into the running accumulator
                w = CF // 2
                while w > 4 * F:
                    nc.vector.tensor_add(prod[:, 0:w], prod[:, 0:w], prod[:, w : 2 * w])
                    w //= 2
                if first:
                    nc.vector.tensor_add(
                        acc[:, :], prod[:, 0 : 4 * F], prod[:, 4 * F : 8 * F]
                    )
                    first = False
                else:
                    nc.vector.tensor_add(
                        prod[:, 0 : 4 * F], prod[:, 0 : 4 * F], prod[:, 4 * F : 8 * F]
                    )
                    nc.vector.tensor_add(acc[:, :], acc[:, :], prod[:, 0 : 4 * F])

            # final: 4*F -> F (fp32 at the end), + hop-0 diagonal
            nc.vector.tensor_add(acc[:, 0 : 2 * F], acc[:, 0 : 2 * F], acc[:, 2 * F : 4 * F])
            nc.vector.tensor_add(accf[:, :], acc[:, 0:F], acc[:, F : 2 * F])
            nc.vector.tensor_add(accf[:, :], accf[:, :], d_sb[:, :])

            nc.sync.dma_start(out=out[:, :], in_=accf[:, :])

    nc.compile()
    _CACHE["nc"] = nc
    return nc


def _chunk_major(xslab):
    """[3, NH, N, F] bf16 -> flat chunk-major: for each chunk c,
    [128 i, 3 k, CJ j, F] contiguous."""
    parts = []
    j0 = 0
    for CJ in CJS:
        blk = xslab[:, :, j0 : j0 + CJ, :]          # [3, NH, CJ, F]
        parts.append(np.ascontiguousarray(blk.transpose(1, 0, 2, 3)).reshape(-1))
        j0 += CJ
    return np.concatenate(parts)


def _make_in_maps(A, X):
    idx = np.arange(NH)
    eye = np.eye(128, dtype=np.float32).astype(BF16_NP)
    Xb = X[:, 1:4].astype(BF16_NP)  # (batch, 3, N, N, F) bf16
    in_maps = []
    for c in range(8):
        b, h = c // 2, c % 2
        lo = h * NH
        xk = _chunk_major(Xb[b, :, lo : lo + NH])
        av = np.ascontiguousarray(A[b, lo : lo + NH, :])
        dv = np.ascontiguousarray(X[b, 0, lo + idx, lo + idx, :])
        in_maps.append({"xk": xk, "a": av, "d": dv, "eye": eye})
    return in_maps


def run(A, X, trace=False, **kw):
    nc = _build_nc()
    in_maps = _make_in_maps(A, X)
    res = run_bass_kernel_spmd(
        nc, in_maps, core_ids=list(range(8)), trace=trace, **kw
    )
    out = np.empty((BATCH, N, F), dtype=np.float32)
    for c in range(8):
        b, h = c // 2, c % 2
        out[b, h * NH : (h + 1) * NH] = res.results[c]["out"]
    return out, res


def kernel(A, X):
    A = np.asarray(A, dtype=np.float32)
    X = np.asarray(X, dtype=np.float32)
    out, _ = run(A, X, trace=False)
    return out


# revision 10
# speedup vs baseline: 1.0353x; 1.0353x over previous
"""GNN message-passing kernel for Trainium2 (8 NeuronCores).

Reference computation:
    out[b,i,f] = X[b,0,i,i,f] + sum_{k=1..3} sum_j A[b,i,j] * X[b,k,i,j,f]

Sharding: 8 cores = (batch b in 0..3) x (i-half h in 0..1); each core owns
a (b, 128-row i-slab) of the output. Hop 0 only contributes its diagonal,
so only X[b,1:4] (3/4 of X) plus the hop-0 diagonal rows are ever sent to
the device. X is converted to bf16 on the host (rel tol is 2e-2; bf16
round-to-nearest gives ~6e-3), halving DMA traffic to ~12.6 MB per core,
and re-laid-out chunk-major so each j-chunk is one fully contiguous DMA.

Per-core device kernel (chunks of CJ j-columns, small tail chunks):
  - One DMA per chunk: [128 part x (3 hops * CJ * F)] contiguous.
  - Hop sum on the TensorEngine: identity-stationary bf16 matmuls (single
    pass) accumulate x1+x2+x3 into PSUM fp32. A warm-up burst plus dense
    chunk cadence keeps HAM at 2.4 GHz (any >3.4us PE gap re-throttles).
  - DVE per chunk: ONE broadcast-AP multiply by A[i,j] (PSUM fp32 src, 1x
    mode) writing bf16, and ONE wide bf16 add (2x mode) into a running
    [128, 2048] accumulator; the j-reduction tree is deferred to a short
    final fold, keeping DVE under the DMA cadence (it is the pacer).
"""

import sys

if "/opt/trn_rl_repo" not in sys.path:
    sys.path.insert(0, "/opt/trn_rl_repo")

import ml_dtypes
import numpy as np

import concourse.bacc as bacc
import concourse.bass as bass
import concourse.mybir as mybir
from concourse.bass_utils import run_bass_kernel_spmd
from concourse.tile import TileContext

BATCH, KP1, N, F = 4, 4, 256, 64
NH = N // 2          # 128 rows of output per core (partition dim)
CJS = [32, 32, 32, 32, 32, 32, 32, 16, 8, 8]   # sum = 256
assert sum(CJS) == N
MMCOL = 512          # moving columns per matmul (ISA max)
ACCW = 2048          # running accumulator width (elements per partition)
FP32 = mybir.dt.float32
BF16 = mybir.dt.bfloat16
BF16_NP = ml_dtypes.bfloat16

_CACHE = {}


def _build_nc():
    if "nc" in _CACHE:
        return _CACHE["nc"]
    nc = bacc.Bacc("TRN2", target_bir_lowering=False, debug=False, num_devices=8)
    # chunk-major: all of chunk c (128 i x 3 hops x CJ j x F) contiguous
    xk = nc.dram_tensor("xk", [NH * 3 * N * F], BF16, kind="ExternalInput").ap()
    a = nc.dram_tensor("a", [NH, N], FP32, kind="ExternalInput").ap()
    d = nc.dram_tensor("d", [NH, F], FP32, kind="ExternalInput").ap()
    eye = nc.dram_tensor("eye", [128, 128], BF16, kind="ExternalInput").ap()
    out = nc.dram_tensor("out", [NH, F], FP32, kind="ExternalOutput").ap()

    with TileContext(nc) as tc:
        with (
            tc.tile_pool(name="const", bufs=1) as cpool,
            tc.tile_pool(name="xs", bufs=4) as xpool,
            tc.tile_pool(name="pr", bufs=2) as prpool,
            tc.tile_pool(name="ac", bufs=1) as acpool,
            tc.tile_pool(name="ps", bufs=2, space="PSUM") as pspool,
        ):
            eye_sb = cpool.tile([128, 128], BF16)
            nc.sync.dma_start(out=eye_sb[:, :], in_=eye[:, :])
            a_sb = cpool.tile([128, N], FP32)
            d_sb = cpool.tile([128, F], FP32)

            acc = acpool.tile([128, ACCW], BF16)
            accf = acpool.tile([128, F], FP32)

            # PE warm-up: bridge from eye-DMA arrival (~7.5us) to chunk-0
            # compute (~11us) with 128-col matmuls so HAM is warm for the
            # real stream.
            warm = pspool.tile([128, max(CJS) * F], FP32, name="ps", tag="ps")
            for _ in range(36):
                nc.tensor.matmul(
                    warm[:, 0:128],
                    eye_sb[:, :],
                    eye_sb[:, :],
                    start=True,
                    stop=True,
                )

            # width of the live acc region while processing chunk c
            live = ACCW
            xoff = 0
            for c, CJ in enumerate(CJS):
                CF = CJ * F
                xt = xpool.tile([128, 3 * CF], BF16, name="xt", tag="xt")
                src = bass.AP(xk.tensor, xoff, [[3 * CF, 128], [1, 3 * CF]])
                nc.sync.dma_start(out=xt[:, :], in_=src)
                xoff += 128 * 3 * CF
                if c == 0:
                    # a/d triggers queue right behind chunk 0's
                    nc.sync.dma_start(out=a_sb[:, :], in_=a[:, :])
                    nc.sync.dma_start(out=d_sb[:, :], in_=d[:, :])

                # hop sum on TensorEngine: bf16 identity matmuls, PSUM fp32
                ps = pspool.tile([128, CF], FP32, name="ps", tag="ps")
                for s in range(CF // MMCOL):
                    sl = slice(s * MMCOL, (s + 1) * MMCOL)
                    for k in range(3):
                        nc.tensor.matmul(
                            ps[:, sl],
                            eye_sb[:, :],
                            xt[:, k * CF + s * MMCOL : k * CF + (s + 1) * MMCOL],
                            start=(k == 0),
                            stop=(k == 2),
                        )

                # prod[i, j*F+f] = ps[i, j*F+f] * a_sb[i, j0+j]  (bf16 out)
                j0 = sum(CJS[:c])
                prod = prpool.tile([128, CF], BF16, name="prod", tag="prod")
                ps_step = ps.ap[0][0]
                pr_step = prod.ap[0][0]
                a_step = a_sb.ap[0][0]
                in0 = bass.AP(ps.tensor, 0, [[ps_step, 128], [F, CJ], [1, F]])
                in1 = bass.AP(a_sb.tensor, j0, [[a_step, 128], [1, CJ], [0, F]])
                po = bass.AP(prod.tensor, 0, [[pr_step, 128], [F, CJ], [1, F]])
                nc.vector.tensor_mul(po, in0, in1)

                # fold the live acc region down before a narrower chunk
                while live > CF:
                    h = live // 2
                    nc.vector.tensor_add(acc[:, 0:h], acc[:, 0:h], acc[:, h:live])
                    live = h
                if c == 0:
                    nc.vector.tensor_copy(acc[:, 0:CF], prod[:, :])
                else:
                    nc.vector.tensor_add(acc[:, 0:CF], acc[:, 0:CF], prod[:, :])

            # final fold: live -> F (fp32 at the end), + hop-0 diagonal
            while live > 2 * F:
                h = live // 2
                nc.vector.tensor_add(acc[:, 0:h], acc[:, 0:h], acc[:, h:live])
                live = h
            nc.vector.tensor_add(accf[:, :], acc[:, 0:F], acc[:, F : 2 * F])
            nc.vector.tensor_add(accf[:, :], accf[:, :], d_sb[:, :])

            nc.sync.dma_start(out=out[:, :], in_=accf[:, :])

    nc.compile()
    _CACHE["nc"] = nc
    return nc


def _chunk_major(xslab):
    """[3, NH, N, F] bf16 -> flat chunk-major: for each chunk c,
    [128 i, 3 k, CJ j, F] contiguous."""
    parts = []
    j0 = 0
    for CJ in CJS:
        blk = xslab[:, :, j0 : j0 + CJ, :]          # [3, NH, CJ, F]
        parts.append(np.ascontiguousarray(blk.transpose(1, 0, 2, 3)).reshape(-1))
        j0 += CJ
    return np.concatenate(parts)


def _make_in_maps(A, X):
    idx = np.arange(NH)
    eye = np.eye(128, dtype=np.float32).astype(BF16_NP)
    Xb = X[:, 1:4].astype(BF16_NP)  # (batch, 3, N, N, F) bf16
    in_maps = []
    for c in range(8):
        b, h = c // 2, c % 2
        lo = h * NH
        xk = _chunk_major(Xb[b, :, lo : lo + NH])
        av = np.ascontiguousarray(A[b, lo : lo + NH, :])
        dv = np.ascontiguousarray(X[b, 0, lo + idx, lo + idx, :])
        in_maps.append({"xk": xk, "a": av, "d": dv, "eye": eye})
    return in_maps


def run(A, X, trace=False, **kw):
    nc = _build_nc()
    in_maps = _make_in_maps(A, X)
    res = run_bass_kernel_spmd(
        nc, in_maps, core_ids=list(range(8)), trace=trace, **kw
    )
    out = np.empty((BATCH, N, F), dtype=np.float32)
    for c in range(8):
        b, h = c // 2, c % 2
        out[b, h * NH : (h + 1) * NH] = res.results[c]["out"]
    return out, res


def kernel(A, X):
    A = np.asarray(A, dtype=np.float32)
    X = np.asarray(X, dtype=np.float32)
    out, _ = run(A, X, trace=False)
    return out


# revision 49
# speedup vs baseline: 1.1080x; 1.0703x over previous
"""GNN message-passing kernel for Trainium2 (8 NeuronCores).

Reference computation:
    out[b,i,f] = X[b,0,i,i,f] + sum_{k=1..3} sum_j A[b,i,j] * X[b,k,i,j,f]

Sharding: 8 cores = (batch b in 0..3) x (i-half h in 0..1); each core owns
a (b, 128-row i-slab) of the output. Hop 0 only contributes its diagonal,
so only X[b,1:4] (3/4 of X) plus the hop-0 diagonal rows are ever sent to
the device. X is converted to bf16 on the host (rel tol is 2e-2; the
full pipeline measures ~7e-3), halving DMA traffic to ~12.6 MB per core,
and re-laid-out chunk-major so each j-chunk is one fully contiguous DMA.
~12.6 MB / ~360 GB/s HBM-per-core is the ~35 us floor this kernel tracks.

Per-core device pipeline (chunks of CJ j-columns; DMA -> PE -> ACT -> DVE):
  - Every chunk has its OWN SBUF buffer (the whole bf16 X slab fits:
    96 KB/partition), so no DMA trigger ever waits on a buffer and the
    stream runs at pure HBM rate regardless of compute hiccups -- this
    decouples the engines and kills backpressure-induced variance.
  - TensorE: identity-stationary bf16 matmuls (single pass) accumulate
    x1+x2+x3 into PSUM fp32. A garbage-weight warm-up burst (memset tile,
    no DMA dependency) trips the HAM clock gate to 2.4 GHz early.
  - ScalarE (otherwise idle): copies the PSUM fp32 hop sum to SBUF bf16.
  - DVE: expands A[i,j] -> expA[i, j*F+f] during the fill (int32
    pair-packed copy, quartered so nothing blocks), then per chunk ONE
    unit-stride bf16 multiply (2x mode; no broadcast AP / PSUM operand,
    either of which would force 1x) and ONE bf16 add into a running
    1024-wide accumulator (+ a fold for 2048-wide chunks); the
    j-reduction finishes in a short final fold.
"""

import sys

if "/opt/trn_rl_repo" not in sys.path:
    sys.path.insert(0, "/opt/trn_rl_repo")

import ml_dtypes
import numpy as np

import concourse.bacc as bacc
import concourse.bass as bass
import concourse.mybir as mybir
from concourse.bass_utils import run_bass_kernel_spmd
from concourse.tile import TileContext

BATCH, KP1, N, F = 4, 4, 256, 64
NH = N // 2          # 128 rows of output per core (partition dim)
CJS = [8, 8, 16] + [32] * 6 + [16, 8, 8]  # sum = 256; small head + tail
assert sum(CJS) == N
MMCOL = 512          # moving columns per matmul (ISA max)
ACCW = 1024          # running accumulator width (elements per partition)
FP32 = mybir.dt.float32
BF16 = mybir.dt.bfloat16
INT32 = mybir.dt.int32
BF16_NP = ml_dtypes.bfloat16

_CACHE = {}


def _build_nc():
    if "nc" in _CACHE:
        return _CACHE["nc"]
    nc = bacc.Bacc("TRN2", target_bir_lowering=False, debug=False, num_devices=8)
    # chunk-major: all of chunk c (128 i x 3 hops x CJ j x F) contiguous
    xk = nc.dram_tensor("xk", [NH * 3 * N * F], BF16, kind="ExternalInput").ap()
    a2 = nc.dram_tensor("a2", [NH, N], INT32, kind="ExternalInput").ap()
    d = nc.dram_tensor("d", [NH, F], FP32, kind="ExternalInput").ap()
    eye = nc.dram_tensor("eye", [128, 128], BF16, kind="ExternalInput").ap()
    out = nc.dram_tensor("out", [NH, F], FP32, kind="ExternalOutput").ap()

    FH = F // 2  # int32 pairs per j in the expanded-A row
    n_small = sum(1 for cj in CJS if cj < 32)
    n_big = sum(1 for cj in CJS if cj >= 32)

    with TileContext(nc) as tc:
        with (
            tc.tile_pool(name="const", bufs=1) as cpool,
            tc.tile_pool(name="xss", bufs=n_small) as xspool,
            tc.tile_pool(name="xsb", bufs=n_big) as xbpool,
            tc.tile_pool(name="sm", bufs=2) as smpool,
            tc.tile_pool(name="pr", bufs=2) as prpool,
            tc.tile_pool(name="ac", bufs=1) as acpool,
            tc.tile_pool(name="ps", bufs=2, space="PSUM") as pspool,
        ):
            # eye/a2 go FIRST on the sync ring so their packets complete
            # before the big chunk transfers occupy the shared SDMA engines.
            eye_sb = cpool.tile([128, 128], BF16)
            nc.sync.dma_start(out=eye_sb[:, :], in_=eye[:, :])
            a2_sb = cpool.tile([128, N], INT32)
            nc.sync.dma_start(out=a2_sb[:, :], in_=a2[:, :])
            d_sb = cpool.tile([128, F], FP32)
            # d is only needed at the end: ACT ring, off the critical path
            nc.scalar.dma_start(out=d_sb[:, :], in_=d[:, :])

            expa = cpool.tile([128, N * FH], INT32)  # = [128, N*F] bf16
            acc = acpool.tile([128, ACCW], BF16)
            nc.vector.memset(acc[:, :], 0.0)
            accf = acpool.tile([128, F], FP32)

            # A-expansion (DVE): expa[i, j*FH + q] = a2[i, j] (int32 =
            # packed bf16 pair), quartered so the first multiply is not
            # blocked behind one long copy.
            e_step = expa.ap[0][0]
            a_step = a2_sb.ap[0][0]

            def expand_quarter(q):
                NQ = N // 4
                eo = bass.AP(
                    expa.tensor, q * NQ * FH, [[e_step, 128], [FH, NQ], [1, FH]]
                )
                ei = bass.AP(
                    a2_sb.tensor, q * NQ, [[a_step, 128], [1, NQ], [0, FH]]
                )
                nc.vector.tensor_copy(eo, ei)

            expand_quarter(0)

            # PE warm-up on a memset tile: no DMA dependency, so the burst
            # starts at engine boot and HAM reaches 2.4 GHz early. Full
            # 128x128 matmuls -- narrow ones don't trip the detector.
            garbage = cpool.tile([128, 128], BF16)
            nc.gpsimd.memset(garbage[:, :], 0.5)
            warm = pspool.tile([128, max(CJS) * F], FP32, name="ps", tag="ps")
            for _ in range(24):
                nc.tensor.matmul(
                    warm[:, 0:128],
                    garbage[:, :],
                    garbage[:, :],
                    start=True,
                    stop=True,
                )

            # DMA granularity is decoupled from compute granularity: the six
            # 32j body chunks arrive as three 64j transfers (fewer DMA
            # fixed-overhead bubbles in the stream); compute still runs in
            # 32j chunks (PSUM-sized), reading halves of the shared tile.
            xoff = 0
            for c, CJ in enumerate(CJS):
                CF = CJ * F
                pool = xbpool if CJ >= 32 else xspool
                xt = pool.tile([128, 3 * CF], BF16, name="xt", tag="xt")
                src = bass.AP(xk.tensor, xoff, [[3 * CF, 128], [1, 3 * CF]])
                nc.sync.dma_start(out=xt[:, :], in_=src)
                xoff += 128 * 3 * CF
                hs, xbase = CF, 0

                # hop sum on TensorEngine: bf16 identity matmuls, PSUM fp32
                ps = pspool.tile([128, CF], FP32, name="ps", tag="ps")
                for s in range(CF // MMCOL):
                    sl = slice(s * MMCOL, (s + 1) * MMCOL)
                    for k in range(3):
                        nc.tensor.matmul(
                            ps[:, sl],
                            eye_sb[:, :],
                            xt[
                                :,
                                xbase + k * hs + s * MMCOL : xbase
                                + k * hs
                                + (s + 1) * MMCOL,
                            ],
                            start=(k == 0),
                            stop=(k == 2),
                        )

                # ScalarE: PSUM fp32 -> SBUF bf16 (frees PSUM, enables 2x mul)
                s_sb = smpool.tile([128, CF], BF16, name="ssb", tag="ssb")
                nc.scalar.copy(s_sb[:, :], ps[:, :])

                # DVE: prod = s_sb * expA[j-slice]  (all bf16 unit-stride, 2x)
                j0 = sum(CJS[:c])
                prod = prpool.tile([128, CF], BF16, name="prod", tag="prod")
                ea = expa[:, j0 * FH : (j0 + CJ) * FH].bitcast(BF16)
                nc.vector.tensor_mul(prod[:, :], s_sb[:, :], ea)

                # fold prod down to the acc width, then accumulate (a
                # narrower prod adds into a prefix -- column sums survive)
                w = CF
                while w > ACCW:
                    h = w // 2
                    nc.vector.tensor_add(prod[:, 0:h], prod[:, 0:h], prod[:, h:w])
                    w = h
                nc.vector.tensor_add(acc[:, 0:w], acc[:, 0:w], prod[:, 0:w])
                if c in (0, 1, 2):
                    expand_quarter(c + 1)
                if c == len(CJS) - 3:
                    # pre-fold while the tail chunks stream: the remaining
                    # (narrow) chunks only add into acc[0:512]
                    nc.vector.tensor_add(
                        acc[:, 0:512], acc[:, 0:512], acc[:, 512:1024]
                    )

            # final fold: 512 -> F (fp32 at the end), + hop-0 diagonal
            live = 512
            while live > 2 * F:
                h = live // 2
                nc.vector.tensor_add(acc[:, 0:h], acc[:, 0:h], acc[:, h:live])
                live = h
            nc.vector.tensor_add(accf[:, :], acc[:, 0:F], acc[:, F : 2 * F])
            nc.vector.tensor_add(accf[:, :], accf[:, :], d_sb[:, :])

            nc.sync.dma_start(out=out[:, :], in_=accf[:, :])

    nc.compile()
    _CACHE["nc"] = nc
    return nc


def _chunk_major(xslab):
    """[3, NH, N, F] bf16 -> flat chunk-major: for each chunk,
    [128 i, 3 k, CJ j, F] contiguous."""
    parts = []
    j0 = 0
    for CJ in CJS:
        blk = xslab[:, :, j0 : j0 + CJ, :]          # [3, NH, CJ, F]
        parts.append(np.ascontiguousarray(blk.transpose(1, 0, 2, 3)).reshape(-1))
        j0 += CJ
    return np.concatenate(parts)


def _make_in_maps(A, X):
    idx = np.arange(NH)
    eye = np.eye(128, dtype=np.float32).astype(BF16_NP)
    Xb = X[:, 1:4].astype(BF16_NP)  # (batch, 3, N, N, F) bf16
    in_maps = []
    for c in range(8):
        b, h = c // 2, c % 2
        lo = h * NH
        xk = _chunk_major(Xb[b, :, lo : lo + NH])
        ab = np.asarray(A[b, lo : lo + NH, :], dtype=np.float32).astype(BF16_NP)
        au = ab.view(np.uint16).astype(np.uint32)
        a2 = ((au << 16) | au).view(np.int32)
        dv = np.ascontiguousarray(X[b, 0, lo + idx, lo + idx, :])
        in_maps.append({"xk": xk, "a2": a2, "d": dv, "eye": eye})
    return in_maps


def run(A, X, trace=False, **kw):
    nc = _build_nc()
    in_maps = _make_in_maps(A, X)
    res = run_bass_kernel_spmd(
        nc, in_maps, core_ids=list(range(8)), trace=trace, **kw
    )
    out = np.empty((BATCH, N, F), dtype=np.float32)
    for c in range(8):
        b, h = c // 2, c % 2
        out[b, h * NH : (h + 1) * NH] = res.results[c]["out"]
    return out, res


def kernel(A, X):
    A = np.asarray(A, dtype=np.float32)
    X = np.asarray(X, dtype=np.float32)
    out, _ = run(A, X, trace=False)
    return out
